# revision 26
# baseline (speedup 1.0000x reference)
"""Trainium2 Bass kernel for nn_CoNe_35974646071945 (retrieval_knn).

Strategy: K-shard the 65536-entry queue across 8 NeuronCores. Host
pre-converts operands (bf16 for the sim matmuls, scaled e4m3 fp8 for the
class-probability matrix). Each core:

  phase 1 (per 128-wide j-tile of its 8192 queue columns):
    pqk[j, 0:512]    = queue_shard^T @ k_feat^T   (bf16 matmul, fp32 PSUM)
    pqk[j, 512:1024] = queue_shard^T @ norm_q^T
    et8[j, t, b] = exp(pqk[:, :512] / T_DC)       (scalar engine, fp8 out)
    simq tile    = fp16(pqk[:, 512:])             (DVE) -> DMA out

  phase 2 (fp8 DoubleRow, two j-tiles contracted per matmul):
    P[b, cls+] += et8_pair^T @ qlp8_pair          (fp32 PSUM accumulation;
                                                   qlp8 col 1000 holds 224
                                                   giving the softmax
                                                   partition Z*224)

Outputs per core: simq fp16 (host does the exact top-200 selection) and
the P partials. Host sums P over cores (the psum), does the top-k /
softmax / KL scalar math on tiny arrays, and returns the 3 losses.

qlp8 DRAM layout is pre-transposed on host to [128, NJT*1008] so each
SBUF partition row is one wide contiguous DMA (the DMA engines are
per-packet-throughput-bound; packet bytes = bytes per partition row).
"""
import sys
sys.path.insert(0, '/opt/trn_rl_repo')
sys.path.insert(0, '/root/.axon_site/_ro/trn_rl_repo')

import numpy as np
import ml_dtypes
from contextlib import ExitStack

from concourse import bass, tile, mybir
from concourse.bass_utils import run_bass_kernel_spmd
from concourse.vector_clock import ScopedClock, VectorClock

F32 = mybir.dt.float32
F16 = mybir.dt.float16
BF16 = mybir.dt.bfloat16
F8E4 = mybir.dt.float8e4

N_CORES = 8
B, D, K, C = 512, 256, 65536, 1000
KS = K // N_CORES            # 8192 queue columns per core
T_SUP, T_DC, LS = 0.07, 0.1, 0.1
EPS = 1e-8
NJT = KS // 128              # 64 j-tiles per core
QLW = 1008                   # padded qlp width: 1000 classes + Z + 7 pad
QLP_SCALE = 1024.0           # fp8 scale for class-probability entries
Z_SCALE = 224.0              # fp8-exact scale for the Z (ones) column
SQ_SCALE = 512.0             # fp8 scale for queue / k_feat sim operands


class CompatTileContext(tile.TileContext):
    """This walrus build encodes at most ONE sync wait per instruction.
    Split Tile's multi-wait instructions and its tail drain."""

    def _commit_instruction(self, inst, lazy_reg_writes=True):
        si = inst.sync_info
        if (
            si is not None
            and si.on_wait
            and len(si.on_wait) > 1
            and inst.engine != mybir.EngineType.Unassigned
        ):
            import bass_rust
            waits = list(si.on_wait)
            for w in waits[:-1]:
                nop = mybir.InstNoOp(
                    name=f"I-{self.nc.next_id()}", ins=[], outs=[]
                )
                nop.engine = inst.engine
                nop.sync_info = bass_rust.SyncInfo(on_wait=[w], on_update=[])
                super()._commit_instruction(nop, lazy_reg_writes=False)
            si.on_wait = [waits[-1]]
            inst.sync_info = si
        super()._commit_instruction(inst, lazy_reg_writes=lazy_reg_writes)

    def _drain_and_barrier(self, tick_clock, wait_clock):
        gclock = tick_clock.global_clock
        n = len(gclock)
        for i in range(n):
            if gclock[i] == 0:
                continue
            vec = [0] * n
            vec[i] = gclock[i]
            nop_inst = self.nc.sync.nop(nofuse=True, hint=f"tail_wait_p{i}")
            wait_clock.add_sem_waits(
                nop_inst.ins, ScopedClock({None: VectorClock(vec)})
            )
        self.nc.sync.drain()
        self.nc.all_engine_barrier()
        assert self.sems is not None
        popped = self.nc._tile_sem_poison_stack.pop()
        assert popped is self._sem_poison
        self.nc.clear_and_free_semaphores(list(self.sems.allocated().values()))
        self.nc.all_engine_barrier()


_CACHED = {}


def _build():
    if 'nc' in _CACHED:
        return _CACHED['nc']
    nc = bass.Bass(num_devices=N_CORES)
    qT_in = nc.declare_dram_parameter("qT", [D, B], BF16, isOutput=False)
    qsh_in = nc.declare_dram_parameter("qsh", [D, KS], BF16, isOutput=False)
    q8_in = nc.declare_dram_parameter("qsh8", [128, 2, KS], F8E4,
                                      isOutput=False)
    k8_in = nc.declare_dram_parameter("kT8", [128, 2, B], F8E4,
                                      isOutput=False)
    qlp_in = nc.declare_dram_parameter("qlp8", [128, NJT * QLW], F8E4,
                                       isOutput=False)
    # p-major layout: row p, col t*512+b holds sim[j=t*128+p, b] — keeps
    # each DMA packet (one SBUF partition row) 8KB wide
    simq_out = nc.declare_dram_parameter("simq", [128, NJT * B], F16,
                                         isOutput=True)
    p_out = nc.declare_dram_parameter("pout", [B, QLW], F32, isOutput=True)

    with ExitStack() as ctx:
        tc = ctx.enter_context(CompatTileContext(nc))
        pool = ctx.enter_context(tc.tile_pool(name="main", bufs=1))
        sq = ctx.enter_context(tc.tile_pool(name="sq", bufs=2))

        # resident operands; queue shards arrive in j-order chunks so the
        # first j-tiles' matmuls start without waiting for the full shard
        # first matmul (fp8 pk of tile 0) needs kT8 + the first q8 chunk:
        # load those ahead of everything else
        kT8 = pool.tile([128, 2, B], F8E4, name="kT8")
        nc.sync.dma_start(kT8[:], k8_in[:])
        q8 = pool.tile([128, 2, KS], F8E4, name="q8")
        QCH = KS // 4
        nc.sync.dma_start(q8[:, :, 0:QCH], q8_in[:, :, 0:QCH])
        qTb = []
        qb = []
        for d in range(2):
            t = pool.tile([128, B], BF16, name=f"qT{d}")
            nc.sync.dma_start(t[:], qT_in[d * 128:(d + 1) * 128, :])
            qTb.append(t)
            qb.append(pool.tile([128, KS], BF16, name=f"qb{d}"))
        for ch in range(4):
            cl = ch * QCH
            if ch > 0:
                nc.sync.dma_start(q8[:, :, cl:cl + QCH],
                                  q8_in[:, :, cl:cl + QCH])
            for d in range(2):
                nc.sync.dma_start(
                    qb[d][:, cl:cl + QCH],
                    qsh_in[d * 128:(d + 1) * 128, cl:cl + QCH])

        et8 = pool.tile([128, NJT, B], F8E4, name="et8")
        qlp8 = pool.tile([128, NJT, QLW], F8E4, name="qlp8")

        GS = 8                               # simq upload group: 8 j-tiles
        DR = mybir.MatmulPerfMode.DoubleRow
        NC1 = 512
        HW2 = QLW // 2
        NPR = NJT // 2                       # 32 tile pairs
        PDL = 4                              # pair delay so P-matmuls never
                                             # race a fresh exp or qlp8 chunk

        def p_mms(pacc, pr, bts):
            """accumulate pair pr of ET^T @ qlp8 into pacc[bt] (fp8 DR)"""
            pl = 2 * pr
            for bt in bts:
                lhs = et8[:, pl:pl + 2, bt * 128:(bt + 1) * 128]
                nc.tensor.matmul(pacc[bt % 2][:, :NC1], lhs,
                                 qlp8[:, pl:pl + 2, :NC1],
                                 start=(pr == 0), stop=(pr == NPR - 1),
                                 perf_mode=DR)
                nc.tensor.matmul(pacc[bt % 2][:, NC1:QLW], lhs,
                                 qlp8[:, pl:pl + 2, NC1:QLW],
                                 start=(pr == 0), stop=(pr == NPR - 1),
                                 perf_mode=DR)

        def p_flush(pacc, bts):
            """PSUM -> SBUF (split DVE/scalar) -> DRAM, per column half"""
            for bt in bts:
                pcp = sq.tile([128, QLW], F32, name="pcp", tag="pcp")
                nc.vector.tensor_copy(pcp[:, :HW2], pacc[bt % 2][:, :HW2])
                nc.scalar.copy(pcp[:, HW2:], pacc[bt % 2][:, HW2:QLW])
                nc.sync.dma_start(p_out[bt * 128:(bt + 1) * 128, :HW2],
                                  pcp[:, :HW2])
                nc.sync.dma_start(p_out[bt * 128:(bt + 1) * 128, HW2:],
                                  pcp[:, HW2:])

        with ExitStack() as phs:
            ps1 = phs.enter_context(
                tc.tile_pool(name="ps1", bufs=2, space="PSUM"))
            ps2 = phs.enter_context(
                tc.tile_pool(name="ps2", bufs=1, space="PSUM"))
            pacc = [ps2.tile([128, 1024], F32, name=f"pacc{i}")
                    for i in range(2)]
            # pass A: phase-1 tiles with bt0/bt1 P-accumulation interleaved
            for t in range(NJT):
                jl = t * 128
                pk = ps1.tile([128, B], F32, name="pk", tag="pk")
                pq = ps1.tile([128, B], F32, name="pq", tag="pq")
                # k_feat sims: one fp8 DoubleRow matmul contracts all of D
                nc.tensor.matmul(pk[:], q8[:, :, jl:jl + 128],
                                 kT8[:], start=True, stop=True,
                                 perf_mode=DR)
                # norm_q sims: bf16 for top-k precision
                for d in range(2):
                    nc.tensor.matmul(
                        pq[:], qb[d][:, jl:jl + 128], qTb[d][:],
                        start=(d == 0), stop=(d == 1))
                nc.scalar.activation(et8[:, t, :], pk[:],
                                     mybir.ActivationFunctionType.Exp,
                                     scale=(1.0 / T_DC) / (SQ_SCALE * SQ_SCALE))
                if t % GS == 0:
                    sqg = sq.tile([128, GS, B], F16, name="sqg", tag="sqg")
                nc.vector.tensor_copy(sqg[:, t % GS, :], pq[:])
                if t == 0:
                    nc.sync.dma_start(qlp8[:, 0:8, :],
                                      qlp_in[:, 0:8 * QLW])
                if t % GS == GS - 1:
                    g = t // GS
                    nc.sync.dma_start(
                        simq_out[:, g * GS * B:(g + 1) * GS * B], sqg[:])
                    # prefetch the NEXT qlp8 chunk behind this group's
                    # upload — a full group ahead of its first consumer
                    if g + 1 < NJT // 8:
                        nc.sync.dma_start(
                            qlp8[:, (g + 1) * 8:(g + 2) * 8, :],
                            qlp_in[:, (g + 1) * 8 * QLW:(g + 2) * 8 * QLW])
                if t % 2 == 1 and t >= 2 * PDL + 1:
                    p_mms(pacc, (t - 1) // 2 - PDL, (0, 1))
            for pr in range(NPR - PDL, NPR):
                p_mms(pacc, pr, (0, 1))
            p_flush(pacc, (0, 1))
        # pass B: pure P-accumulation for bt2/bt3 in fresh PSUM banks
        # (reusing pass A's pacc raced the in-flight flush reads)
        with ExitStack() as phb:
            ps3 = phb.enter_context(
                tc.tile_pool(name="ps3", bufs=1, space="PSUM"))
            pacc_b = [ps3.tile([128, 1024], F32, name=f"paccb{i}")
                      for i in range(2)]
            for pr in range(NPR):
                p_mms(pacc_b, pr, (2, 3))
            p_flush(pacc_b, (2, 3))

    _CACHED['nc'] = nc
    return nc


def _in_maps(norm_q, k_feat, queue, qlp):
    """Host-side shard + dtype conversion. All args float32 full arrays."""
    qT = np.ascontiguousarray(norm_q.T).astype(ml_dtypes.bfloat16)
    kT8 = np.ascontiguousarray(
        (k_feat.T * SQ_SCALE).astype(ml_dtypes.float8_e4m3)
        .reshape(2, 128, B).transpose(1, 0, 2))                 # [128, 2, B]
    queue_b = queue.astype(ml_dtypes.bfloat16)                  # [D, K]
    queue_8 = (queue * SQ_SCALE).astype(ml_dtypes.float8_e4m3)  # [D, K]
    # fp8 qlp, transposed + scaled + augmented, laid out so that DRAM row p
    # holds [tile t, col c] = qlp[c, t*128+p] for the owning core
    qlpT = np.ascontiguousarray(qlp.T) * QLP_SCALE              # [K, C]
    aug = np.empty((K, QLW), np.float32)
    aug[:, :C] = qlpT
    aug[:, C] = Z_SCALE
    aug[:, C + 1:] = 0.0
    aug8 = aug.astype(ml_dtypes.float8_e4m3)                    # [K, QLW]
    in_maps = []
    for c in range(N_CORES):
        sh = slice(c * KS, (c + 1) * KS)
        blk = aug8[sh].reshape(NJT, 128, QLW).transpose(1, 0, 2)
        in_maps.append({
            "qT": qT,
            "kT8": kT8,
            "qsh": np.ascontiguousarray(queue_b[:, sh]),
            "qsh8": np.ascontiguousarray(
                queue_8[:, sh].reshape(2, 128, KS).transpose(1, 0, 2)),
            "qlp8": np.ascontiguousarray(blk).reshape(128, NJT * QLW),
        })
    return in_maps


def kernel(norm_q, q_logits, k_feat, logits_k, queue, queue_label_prob,
           queue_label, target, knn_k):
    norm_q = np.asarray(norm_q, np.float32)
    q_logits = np.asarray(q_logits, np.float32)
    k_feat = np.asarray(k_feat, np.float32)
    queue = np.asarray(queue, np.float32)
    qlp = np.asarray(queue_label_prob, np.float32)
    queue_label = np.asarray(queue_label)
    target = np.asarray(target)
    kk = int(knn_k)

    nc = _build()
    res = run_bass_kernel_spmd(nc, _in_maps(norm_q, k_feat, queue, qlp),
                               list(range(N_CORES)))

    sim = np.concatenate(
        [res.results[c]["simq"].reshape(128, NJT, B)
         .transpose(1, 0, 2).reshape(KS, B).T.astype(np.float32)
         for c in range(N_CORES)], axis=1)
    P = np.zeros((B, QLW), np.float64)
    for c in range(N_CORES):
        P += res.results[c]["pout"].astype(np.float64)

    # ---- supcon (exact top-k on the device-computed sim) ----
    idx = np.argpartition(-sim, kk - 1, axis=1)[:, :kk]
    sim_knn = np.take_along_axis(sim, idx, axis=1)
    w = np.exp((sim_knn - sim_knn.max(axis=1, keepdims=True)) / T_SUP)
    w /= w.sum(axis=1, keepdims=True)
    pos = (target[:, None] == queue_label[idx])
    gt = (w * pos).sum(axis=1)
    m = gt > EPS
    supin_loss = np.where(m, -np.log(np.where(m, gt, 1.0)), 0.0).sum() / B

    # ---- fc loss ----
    x = q_logits.astype(np.float64)
    lse = np.log(np.exp(x - x.max(1, keepdims=True)).sum(1)) + x.max(1)
    log_q = x - lse[:, None]
    q_mask = (x.min(1) - lse) > np.log(EPS)
    onehot = np.full((B, C), LS / (C - 1))
    onehot[np.arange(B), target] = 1.0 - LS
    fc_loss = -((onehot * log_q).sum(1) * q_mask).sum() / B

    # ---- dc loss ----
    Z = P[:, C] / Z_SCALE
    dc_t = (P[:, :C] / QLP_SCALE) / Z[:, None]
    dc_pos = dc_t > 0
    kl = np.where(dc_pos,
                  dc_t * (np.log(np.where(dc_pos, dc_t, 1.0)) - log_q), 0.0)
    dc_loss = (kl.sum(1) * q_mask).sum() / B

    return (np.float32(supin_loss), np.float32(fc_loss), np.float32(dc_loss))


# revision 28
# speedup vs baseline: 1.0057x; 1.0057x over previous
"""Trainium2 Bass kernel for nn_CoNe_35974646071945 (retrieval_knn).

Strategy: K-shard the 65536-entry queue across 8 NeuronCores. Host
pre-converts operands (bf16 for the sim matmuls, scaled e4m3 fp8 for the
class-probability matrix). Each core:

  phase 1 (per 128-wide j-tile of its 8192 queue columns):
    pqk[j, 0:512]    = queue_shard^T @ k_feat^T   (bf16 matmul, fp32 PSUM)
    pqk[j, 512:1024] = queue_shard^T @ norm_q^T
    et8[j, t, b] = exp(pqk[:, :512] / T_DC)       (scalar engine, fp8 out)
    simq tile    = fp16(pqk[:, 512:])             (DVE) -> DMA out

  phase 2 (fp8 DoubleRow, two j-tiles contracted per matmul):
    P[b, cls+] += et8_pair^T @ qlp8_pair          (fp32 PSUM accumulation;
                                                   qlp8 col 1000 holds 224
                                                   giving the softmax
                                                   partition Z*224)

Outputs per core: simq fp16 (host does the exact top-200 selection) and
the P partials. Host sums P over cores (the psum), does the top-k /
softmax / KL scalar math on tiny arrays, and returns the 3 losses.

qlp8 DRAM layout is pre-transposed on host to [128, NJT*1008] so each
SBUF partition row is one wide contiguous DMA (the DMA engines are
per-packet-throughput-bound; packet bytes = bytes per partition row).
"""
import sys
sys.path.insert(0, '/opt/trn_rl_repo')
sys.path.insert(0, '/root/.axon_site/_ro/trn_rl_repo')

import numpy as np
import ml_dtypes
from contextlib import ExitStack

from concourse import bass, tile, mybir
from concourse.bass_utils import run_bass_kernel_spmd
from concourse.vector_clock import ScopedClock, VectorClock

F32 = mybir.dt.float32
F16 = mybir.dt.float16
BF16 = mybir.dt.bfloat16
F8E4 = mybir.dt.float8e4

N_CORES = 8
B, D, K, C = 512, 256, 65536, 1000
KS = K // N_CORES            # 8192 queue columns per core
T_SUP, T_DC, LS = 0.07, 0.1, 0.1
EPS = 1e-8
NJT = KS // 128              # 64 j-tiles per core
QLW = 1008                   # padded qlp width: 1000 classes + Z + 7 pad
QLP_SCALE = 1024.0           # fp8 scale for class-probability entries
Z_SCALE = 224.0              # fp8-exact scale for the Z (ones) column
SQ_SCALE = 512.0             # fp8 scale for queue / k_feat sim operands


class CompatTileContext(tile.TileContext):
    """This walrus build encodes at most ONE sync wait per instruction.
    Split Tile's multi-wait instructions and its tail drain."""

    def _commit_instruction(self, inst, lazy_reg_writes=True):
        si = inst.sync_info
        if (
            si is not None
            and si.on_wait
            and len(si.on_wait) > 1
            and inst.engine != mybir.EngineType.Unassigned
        ):
            import bass_rust
            waits = list(si.on_wait)
            for w in waits[:-1]:
                nop = mybir.InstNoOp(
                    name=f"I-{self.nc.next_id()}", ins=[], outs=[]
                )
                nop.engine = inst.engine
                nop.sync_info = bass_rust.SyncInfo(on_wait=[w], on_update=[])
                super()._commit_instruction(nop, lazy_reg_writes=False)
            si.on_wait = [waits[-1]]
            inst.sync_info = si
        super()._commit_instruction(inst, lazy_reg_writes=lazy_reg_writes)

    def _drain_and_barrier(self, tick_clock, wait_clock):
        gclock = tick_clock.global_clock
        n = len(gclock)
        for i in range(n):
            if gclock[i] == 0:
                continue
            vec = [0] * n
            vec[i] = gclock[i]
            nop_inst = self.nc.sync.nop(nofuse=True, hint=f"tail_wait_p{i}")
            wait_clock.add_sem_waits(
                nop_inst.ins, ScopedClock({None: VectorClock(vec)})
            )
        self.nc.sync.drain()
        self.nc.all_engine_barrier()
        assert self.sems is not None
        popped = self.nc._tile_sem_poison_stack.pop()
        assert popped is self._sem_poison
        self.nc.clear_and_free_semaphores(list(self.sems.allocated().values()))
        self.nc.all_engine_barrier()


_CACHED = {}


def _build():
    if 'nc' in _CACHED:
        return _CACHED['nc']
    nc = bass.Bass(num_devices=N_CORES)
    qT_in = nc.declare_dram_parameter("qT", [D, B], BF16, isOutput=False)
    qsh_in = nc.declare_dram_parameter("qsh", [D, KS], BF16, isOutput=False)
    q8_in = nc.declare_dram_parameter("qsh8", [128, 2, KS], F8E4,
                                      isOutput=False)
    k8_in = nc.declare_dram_parameter("kT8", [128, 2, B], F8E4,
                                      isOutput=False)
    qlp_in = nc.declare_dram_parameter("qlp8", [128, NJT * QLW], F8E4,
                                       isOutput=False)
    # p-major layout: row p, col t*512+b holds sim[j=t*128+p, b] — keeps
    # each DMA packet (one SBUF partition row) 8KB wide
    simq_out = nc.declare_dram_parameter("simq", [128, NJT * B], F16,
                                         isOutput=True)
    p_out = nc.declare_dram_parameter("pout", [B, QLW], F32, isOutput=True)

    with ExitStack() as ctx:
        tc = ctx.enter_context(CompatTileContext(nc))
        pool = ctx.enter_context(tc.tile_pool(name="main", bufs=1))
        sq = ctx.enter_context(tc.tile_pool(name="sq", bufs=2))

        # resident operands; queue shards arrive in j-order chunks so the
        # first j-tiles' matmuls start without waiting for the full shard
        # first matmul (fp8 pk of tile 0) needs kT8 + the first q8 chunk:
        # load those ahead of everything else
        kT8 = pool.tile([128, 2, B], F8E4, name="kT8")
        nc.sync.dma_start(kT8[:], k8_in[:])
        q8 = pool.tile([128, 2, KS], F8E4, name="q8")
        # graduated j-chunks: small first chunk unblocks tile 0 early
        QBND = [0, 1024, 2560, 4608, KS]
        nc.sync.dma_start(q8[:, :, 0:QBND[1]], q8_in[:, :, 0:QBND[1]])
        qTb = []
        qb = []
        for d in range(2):
            t = pool.tile([128, B], BF16, name=f"qT{d}")
            nc.sync.dma_start(t[:], qT_in[d * 128:(d + 1) * 128, :])
            qTb.append(t)
            qb.append(pool.tile([128, KS], BF16, name=f"qb{d}"))
        for ch in range(4):
            cl, cr = QBND[ch], QBND[ch + 1]
            if ch > 0:
                nc.sync.dma_start(q8[:, :, cl:cr], q8_in[:, :, cl:cr])
            for d in range(2):
                nc.sync.dma_start(
                    qb[d][:, cl:cr],
                    qsh_in[d * 128:(d + 1) * 128, cl:cr])

        et8 = pool.tile([128, NJT, B], F8E4, name="et8")
        qlp8 = pool.tile([128, NJT, QLW], F8E4, name="qlp8")

        GS = 8                               # simq upload group: 8 j-tiles
        with ExitStack() as ph1:
            ps1 = ph1.enter_context(
                tc.tile_pool(name="ps1", bufs=4, space="PSUM"))
            DR = mybir.MatmulPerfMode.DoubleRow
            for t in range(NJT):
                jl = t * 128
                # independent single-bank tiles so the exp and the fp16
                # cast recycle their PSUM buffers independently
                pk = ps1.tile([128, B], F32, name="pk", tag="pk")
                pq = ps1.tile([128, B], F32, name="pq", tag="pq")
                # k_feat sims: one fp8 DoubleRow matmul contracts all of D
                nc.tensor.matmul(pk[:], q8[:, :, jl:jl + 128],
                                 kT8[:], start=True, stop=True,
                                 perf_mode=DR)
                # norm_q sims: bf16 for top-k precision
                for d in range(2):
                    nc.tensor.matmul(
                        pq[:], qb[d][:, jl:jl + 128], qTb[d][:],
                        start=(d == 0), stop=(d == 1))
                nc.scalar.activation(et8[:, t, :], pk[:],
                                     mybir.ActivationFunctionType.Exp,
                                     scale=(1.0 / T_DC) / (SQ_SCALE * SQ_SCALE))
                if t % GS == 0:
                    sqg = sq.tile([128, GS, B], F16, name="sqg", tag="sqg")
                nc.vector.tensor_copy(sqg[:, t % GS, :], pq[:])
                if t % GS == GS - 1:
                    g = t // GS
                    nc.sync.dma_start(
                        simq_out[:, g * GS * B:(g + 1) * GS * B], sqg[:])
                    # interleave qlp8 prefetch behind this group's upload
                    nc.sync.dma_start(
                        qlp8[:, g * 8:(g + 1) * 8, :],
                        qlp_in[:, g * 8 * QLW:(g + 1) * 8 * QLW])

        # phase 2: P[b, cls+] = ET^T @ qlp8, fp8 DoubleRow over tile pairs
        DR = mybir.MatmulPerfMode.DoubleRow
        NC1 = 512
        with ExitStack() as ph2:
            ps2 = ph2.enter_context(
                tc.tile_pool(name="ps2", bufs=1, space="PSUM"))
            pacc = [ps2.tile([128, 1024], F32, name=f"pacc{bt}")
                    for bt in range(4)]
            for pr in range(NJT // 2):
                pl = 2 * pr
                for bt in range(4):
                    lhs = et8[:, pl:pl + 2, bt * 128:(bt + 1) * 128]
                    nc.tensor.matmul(pacc[bt][:, :NC1], lhs,
                                     qlp8[:, pl:pl + 2, :NC1],
                                     start=(pr == 0),
                                     stop=(pr == NJT // 2 - 1),
                                     perf_mode=DR)
                    nc.tensor.matmul(pacc[bt][:, NC1:QLW], lhs,
                                     qlp8[:, pl:pl + 2, NC1:QLW],
                                     start=(pr == 0),
                                     stop=(pr == NJT // 2 - 1),
                                     perf_mode=DR)
            # flush in column halves, alternating DVE/scalar, DMA per half
            HW2 = QLW // 2
            for bt in range(4):
                pcp = sq.tile([128, QLW], F32, name="pcp", tag="pcp")
                nc.vector.tensor_copy(pcp[:, :HW2], pacc[bt][:, :HW2])
                nc.scalar.copy(pcp[:, HW2:], pacc[bt][:, HW2:QLW])
                nc.sync.dma_start(p_out[bt * 128:(bt + 1) * 128, :HW2],
                                  pcp[:, :HW2])
                nc.sync.dma_start(p_out[bt * 128:(bt + 1) * 128, HW2:],
                                  pcp[:, HW2:])

    _CACHED['nc'] = nc
    return nc


def _in_maps(norm_q, k_feat, queue, qlp):
    """Host-side shard + dtype conversion. All args float32 full arrays."""
    qT = np.ascontiguousarray(norm_q.T).astype(ml_dtypes.bfloat16)
    kT8 = np.ascontiguousarray(
        (k_feat.T * SQ_SCALE).astype(ml_dtypes.float8_e4m3)
        .reshape(2, 128, B).transpose(1, 0, 2))                 # [128, 2, B]
    queue_b = queue.astype(ml_dtypes.bfloat16)                  # [D, K]
    queue_8 = (queue * SQ_SCALE).astype(ml_dtypes.float8_e4m3)  # [D, K]
    # fp8 qlp, transposed + scaled + augmented, laid out so that DRAM row p
    # holds [tile t, col c] = qlp[c, t*128+p] for the owning core
    qlpT = np.ascontiguousarray(qlp.T) * QLP_SCALE              # [K, C]
    aug = np.empty((K, QLW), np.float32)
    aug[:, :C] = qlpT
    aug[:, C] = Z_SCALE
    aug[:, C + 1:] = 0.0
    aug8 = aug.astype(ml_dtypes.float8_e4m3)                    # [K, QLW]
    in_maps = []
    for c in range(N_CORES):
        sh = slice(c * KS, (c + 1) * KS)
        blk = aug8[sh].reshape(NJT, 128, QLW).transpose(1, 0, 2)
        in_maps.append({
            "qT": qT,
            "kT8": kT8,
            "qsh": np.ascontiguousarray(queue_b[:, sh]),
            "qsh8": np.ascontiguousarray(
                queue_8[:, sh].reshape(2, 128, KS).transpose(1, 0, 2)),
            "qlp8": np.ascontiguousarray(blk).reshape(128, NJT * QLW),
        })
    return in_maps


def kernel(norm_q, q_logits, k_feat, logits_k, queue, queue_label_prob,
           queue_label, target, knn_k):
    norm_q = np.asarray(norm_q, np.float32)
    q_logits = np.asarray(q_logits, np.float32)
    k_feat = np.asarray(k_feat, np.float32)
    queue = np.asarray(queue, np.float32)
    qlp = np.asarray(queue_label_prob, np.float32)
    queue_label = np.asarray(queue_label)
    target = np.asarray(target)
    kk = int(knn_k)

    nc = _build()
    res = run_bass_kernel_spmd(nc, _in_maps(norm_q, k_feat, queue, qlp),
                               list(range(N_CORES)))

    sim = np.concatenate(
        [res.results[c]["simq"].reshape(128, NJT, B)
         .transpose(1, 0, 2).reshape(KS, B).T.astype(np.float32)
         for c in range(N_CORES)], axis=1)
    P = np.zeros((B, QLW), np.float64)
    for c in range(N_CORES):
        P += res.results[c]["pout"].astype(np.float64)

    # ---- supcon (exact top-k on the device-computed sim) ----
    idx = np.argpartition(-sim, kk - 1, axis=1)[:, :kk]
    sim_knn = np.take_along_axis(sim, idx, axis=1)
    w = np.exp((sim_knn - sim_knn.max(axis=1, keepdims=True)) / T_SUP)
    w /= w.sum(axis=1, keepdims=True)
    pos = (target[:, None] == queue_label[idx])
    gt = (w * pos).sum(axis=1)
    m = gt > EPS
    supin_loss = np.where(m, -np.log(np.where(m, gt, 1.0)), 0.0).sum() / B

    # ---- fc loss ----
    x = q_logits.astype(np.float64)
    lse = np.log(np.exp(x - x.max(1, keepdims=True)).sum(1)) + x.max(1)
    log_q = x - lse[:, None]
    q_mask = (x.min(1) - lse) > np.log(EPS)
    onehot = np.full((B, C), LS / (C - 1))
    onehot[np.arange(B), target] = 1.0 - LS
    fc_loss = -((onehot * log_q).sum(1) * q_mask).sum() / B

    # ---- dc loss ----
    Z = P[:, C] / Z_SCALE
    dc_t = (P[:, :C] / QLP_SCALE) / Z[:, None]
    dc_pos = dc_t > 0
    kl = np.where(dc_pos,
                  dc_t * (np.log(np.where(dc_pos, dc_t, 1.0)) - log_q), 0.0)
    dc_loss = (kl.sum(1) * q_mask).sum() / B

    return (np.float32(supin_loss), np.float32(fc_loss), np.float32(dc_loss))
